# revision 29
# baseline (speedup 1.0000x reference)
"""Bass/Trainium2 kernel for nn_CLUBForCategorical (8-core SPMD).

Math: with lp = log_softmax(x @ W.T + b, axis=-1),
    positive = mean_i lp[i, labels[i]]
    negative = mean_j (mean_i lp)[labels[j]]
    out      = positive - negative

Because lp[i, l] = logits[i, l] - lse_i, the lse_i terms (and the bias b)
cancel exactly in positive - negative:

    out = (1/N) * sum_i x_i . W[labels_i]  -  (1/N^2) * xsum . Sg
    xsum = sum_i x_i,   Sg = sum_j W[labels_j]

so no logits matrix / softmax is needed at all (verified to 2e-13 vs the
full f64 softmax reference, including nonzero b). Per core (batch shard
of 2048 rows): gather W rows at the local labels (SWDGE dma_gather),
row-dot against x on the vector engine (scalar_tensor_tensor with
accum_out), column-sum the gathered W via fp32 ones-matmuls on the
tensor engine and x via a DVE accumulation tree, then one 8-core
AllReduce of [xsum | Sg | diag] (8KB) and a final 1024-wide dot.
Everything is fp32; the kernel is DMA-bound (~16MB/core HBM traffic).
"""

import sys

import numpy as np

if "/opt/trn_rl_repo" not in sys.path:
    sys.path.insert(0, "/opt/trn_rl_repo")

N, D, L = 16384, 1024, 1000
CORES = 8
NLOC = N // CORES          # 2048 rows per core
CHUNK = 256                # rows per DMA/gather chunk
NCHUNK = NLOC // CHUNK     # 8
TPC = CHUNK // 128         # 128-row tiles per chunk = 2
NTILES = NLOC // 128       # 16
CC_LEN = 2056              # xsum[0:1024] | Sg[1024:2048] | diag[2048] | pad

_CACHE: dict = {}


def _build_nc(chunk=CHUNK, small_dma_engine="scalar", big_bufs=4,
              n_acc=1, n_q=1):
    import concourse.bacc as bacc
    import concourse.bass as bass
    import concourse.mybir as mybir
    import concourse.tile as tile
    from concourse import library_config
    from concourse.bass import _add_dep_helper

    nchunk = NLOC // chunk
    tpc = chunk // 128

    f32 = mybir.dt.float32
    i16 = mybir.dt.int16
    add = mybir.AluOpType.add
    mult = mybir.AluOpType.mult
    bypass = mybir.AluOpType.bypass
    X = mybir.AxisListType.X

    nc = bacc.Bacc(
        "TRN2",
        target_bir_lowering=False,
        debug=False,
        num_devices=CORES,
        num_swdge_queues=n_q,
    )
    # x arrives host-pre-tiled: x_tiled[p, t, d] = x_shard[t*128 + p, d],
    # flattened to [128, NTILES*D] so each partition's chunk slice is one
    # contiguous 16KB DMA descriptor.
    x_d = nc.dram_tensor("x", [128, NTILES * D], f32, kind="ExternalInput")
    w_d = nc.dram_tensor("w", [L, D], f32, kind="ExternalInput")
    lidx_d = nc.dram_tensor("lidx", [128, NLOC // 16], i16, kind="ExternalInput")
    out_d = nc.dram_tensor("out", [1, 1], f32, kind="ExternalOutput")

    # chunk view of x: [c][p, t, d] with t the tile-in-chunk index
    x_ch = x_d[:].rearrange("p (c t d) -> c p t d", c=nchunk, t=tpc)

    sdma = nc.scalar if small_dma_engine == "scalar" else nc.sync

    with tile.TileContext(nc) as tc:
        with (
            tc.tile_pool(name="big", bufs=big_bufs) as big,
            tc.tile_pool(name="small", bufs=1) as small,
            tc.tile_pool(name="scratch", bufs=2) as scratch,
            tc.tile_pool(name="ps", bufs=1, space="PSUM") as ps,
            tc.tile_pool(name="dram", bufs=1, space="DRAM") as dram,
        ):
            lib_inst = nc.gpsimd.load_library(library_config.mlp)

            ones = small.tile([128, 1], f32, tag="ones")
            nc.gpsimd.memset(ones[:], 1.0)
            lidx = small.tile([128, NLOC // 16], i16, tag="lidx")
            sdma.dma_start(out=lidx[:], in_=lidx_d[:])

            diag_parts = small.tile([128, NTILES], f32, tag="dparts")
            x_accs = [small.tile([128, D], f32, tag=f"xacc{a}", name=f"xacc{a}")
                      for a in range(n_acc)]

            p_xs0 = ps.tile([1, 512], f32, tag="p_xs0")
            p_xs1 = ps.tile([1, 512], f32, tag="p_xs1")
            p_sg0s = [ps.tile([1, 512], f32, tag=f"p_sg0{a}", name=f"p_sg0{a}")
                      for a in range(n_acc)]
            p_sg1s = [ps.tile([1, 512], f32, tag=f"p_sg1{a}", name=f"p_sg1{a}")
                      for a in range(n_acc)]
            p_dg = ps.tile([1, 1], f32, tag="p_dg")

            ipc = chunk // 16  # idx columns per chunk

            for c in range(nchunk):
                x_c = big.tile([128, tpc, D], f32, tag="x")
                wg_c = big.tile([128, tpc, D], f32, tag="wg")
                nc.sync.dma_start(out=x_c[:], in_=x_ch[c])
                g = nc.gpsimd.dma_gather(
                    wg_c[:],
                    w_d[:],
                    lidx[:, c * ipc : (c + 1) * ipc],
                    chunk,
                    chunk,
                    D,
                    queue_num=c % n_q,
                )
                _add_dep_helper(g.ins, lib_inst.ins, sync=True,
                                reason="mlp library before dma_gather")

                a = c % n_acc
                x_acc = x_accs[a]
                p_sg0, p_sg1 = p_sg0s[a], p_sg1s[a]
                for i in range(tpc):
                    t = c * tpc + i
                    # chain position within this accumulator's chunk stripe
                    afirst = (c < n_acc) and (i == 0)
                    alast = (c >= nchunk - n_acc) and (i == tpc - 1)
                    xt = x_c[:, i, :]
                    wt = wg_c[:, i, :]
                    prod = scratch.tile([128, D], f32, tag="prod")
                    nc.vector.scalar_tensor_tensor(
                        out=prod[:],
                        in0=xt,
                        scalar=1.0,
                        in1=wt,
                        op0=bypass,
                        op1=mult,
                        accum_out=diag_parts[:, t : t + 1],
                    )
                    # xsum: elementwise fp32 tile accumulation on DVE
                    if afirst:
                        nc.vector.tensor_copy(x_acc[:], xt)
                    else:
                        nc.vector.tensor_add(x_acc[:], x_acc[:], xt)
                    # Sg: fp32 ones-matmul column sums (PE has slack)
                    nc.tensor.matmul(p_sg0[:], ones[:], wt[:, 0:512],
                                     start=afirst, stop=alast)
                    nc.tensor.matmul(p_sg1[:], ones[:], wt[:, 512:1024],
                                     start=afirst, stop=alast)

            # combine split accumulators, then xsum partition-reduce (fp32)
            for a in range(1, n_acc):
                nc.vector.tensor_add(x_accs[0][:], x_accs[0][:], x_accs[a][:])
            nc.tensor.matmul(p_xs0[:], ones[:], x_accs[0][:, 0:512],
                             start=True, stop=True)
            nc.tensor.matmul(p_xs1[:], ones[:], x_accs[0][:, 512:1024],
                             start=True, stop=True)

            # reduce per-tile row-dot partials to a single scalar (fp32 path)
            diag_red = small.tile([128, 1], f32, tag="dred")
            nc.vector.tensor_reduce(out=diag_red[:], in_=diag_parts[:], axis=X, op=add)
            nc.tensor.matmul(p_dg[:], diag_red[:], ones[:], start=True, stop=True)

            # assemble the 8KB AllReduce payload on partition 0,
            # pre-scaling by 1/N so the post-AR math is dot + subtract:
            # ans = diag/N - (xsum/N).(Sg/N)
            inv_n = 1.0 / float(N)
            asm = small.tile([1, CC_LEN], f32, tag="asm")
            nc.gpsimd.memset(asm[:], 0.0)
            nc.vector.tensor_scalar_mul(asm[0:1, 0:512], p_xs0[:], inv_n)
            nc.vector.tensor_scalar_mul(asm[0:1, 512:1024], p_xs1[:], inv_n)
            nc.vector.tensor_scalar_mul(asm[0:1, 1024:1536], p_sg0s[0][:], inv_n)
            nc.vector.tensor_scalar_mul(asm[0:1, 1536:2048], p_sg1s[0][:], inv_n)
            sg_tmp = small.tile([1, 512], f32, tag="sgtmp")
            for a in range(1, n_acc):
                nc.vector.tensor_scalar_mul(sg_tmp[:], p_sg0s[a][:], inv_n)
                nc.vector.tensor_add(asm[0:1, 1024:1536],
                                     asm[0:1, 1024:1536], sg_tmp[:])
                nc.vector.tensor_scalar_mul(sg_tmp[:], p_sg1s[a][:], inv_n)
                nc.vector.tensor_add(asm[0:1, 1536:2048],
                                     asm[0:1, 1536:2048], sg_tmp[:])
            nc.vector.tensor_scalar_mul(asm[0:1, 2048:2049], p_dg[:], inv_n)

            cc_in = dram.tile([1, CC_LEN], f32, tag="cc_in")
            cc_out = dram.tile([1, CC_LEN], f32, tag="cc_out")
            sdma.dma_start(out=cc_in[:], in_=asm[:])
            nc.gpsimd.collective_compute(
                "AllReduce",
                add,
                replica_groups=[list(range(CORES))],
                ins=[cc_in[:].opt()],
                outs=[cc_out[:].opt()],
            )
            asm_g = small.tile([1, CC_LEN], f32, tag="asmg")
            sdma.dma_start(out=asm_g[:], in_=cc_out[:])

            # ans = diag/N - dot(xsum, Sg)/N^2
            dotp = small.tile([1, 1024], f32, tag="dotp")
            dotv = small.tile([1, 1], f32, tag="dotv")
            nc.vector.scalar_tensor_tensor(
                out=dotp[:],
                in0=asm_g[0:1, 0:1024],
                scalar=1.0,
                in1=asm_g[0:1, 1024:2048],
                op0=bypass,
                op1=mult,
                accum_out=dotv[:],
            )
            ans = small.tile([1, 1], f32, tag="ans")
            nc.vector.tensor_sub(ans[:], asm_g[0:1, 2048:2049], dotv[:])
            sdma.dma_start(out=out_d[:], in_=ans[:])

    nc.compile()
    return nc


def _get_nc():
    if "nc" not in _CACHE:
        _CACHE["nc"] = _build_nc()
    return _CACHE["nc"]


def _tile_x(x_shard: np.ndarray) -> np.ndarray:
    # [NLOC, D] -> [128, NTILES*D] with x_tiled[p, t*D:(t+1)*D] = x[128t+p]
    return np.ascontiguousarray(
        x_shard.reshape(NTILES, 128, D).transpose(1, 0, 2).reshape(128, NTILES * D)
    )


def _make_lidx(labels_shard: np.ndarray) -> np.ndarray:
    # dma_gather index layout: idx j lives at [j % 16, j // 16], replicated
    # across the 8 gpsimd cores (16-partition groups).
    arr = labels_shard.astype(np.int16).reshape(NLOC // 16, 16).T  # [16, S]
    return np.tile(arr, (8, 1))  # [128, S]


_RUN_KW: dict = {}   # test harness may set e.g. {"trace": True}
LAST_RESULT = None   # BassKernelResults of the most recent run


def kernel(inputs, labels, W, b):
    global LAST_RESULT
    import os

    # The run path needs the axon trn2 PJRT backend; drop a cpu pin if jax
    # hasn't been initialized yet (the reference is jax-on-cpu friendly).
    if "jax" not in sys.modules and os.environ.get("JAX_PLATFORMS") == "cpu":
        del os.environ["JAX_PLATFORMS"]

    from concourse.bass_utils import run_bass_kernel_spmd

    x = np.ascontiguousarray(np.asarray(inputs, dtype=np.float32))
    lab = np.asarray(labels).astype(np.int64)
    w = np.ascontiguousarray(np.asarray(W, dtype=np.float32))
    assert x.shape == (N, D) and w.shape == (L, D) and lab.shape == (N,)
    assert lab.min() >= 0 and lab.max() < L

    nc = _get_nc()
    in_maps = []
    for c in range(CORES):
        sl = slice(c * NLOC, (c + 1) * NLOC)
        in_maps.append(
            {
                "x": _tile_x(x[sl]),
                "w": w,
                "lidx": _make_lidx(lab[sl]),
            }
        )
    res = run_bass_kernel_spmd(nc, in_maps, list(range(CORES)), **_RUN_KW)
    LAST_RESULT = res
    out = np.float32(res.results[0]["out"][0, 0])
    return np.asarray(out, dtype=np.float32)


if __name__ == "__main__":
    import reference

    inp = reference.setup_inputs()
    expected = np.asarray(reference.reference(**inp))
    actual = kernel(**{k: np.asarray(v) for k, v in inp.items()})
    rel = abs(float(actual) - float(expected)) / max(abs(float(expected)), 1e-30)
    print("expected:", expected, "actual:", actual, "rel err:", rel)
